# revision 10
# baseline (speedup 1.0000x reference)
"""Trainium2 Bass kernel for nn_MultiHeadAttention_62371515073076 (v2).

Math (per batch b, faithful to the reference's quirky softmax over the QUERY axis):
  q/k/v = einsum('nc,chd->nhd', x, W{q,k,v})
  s[i,j,h] = q[i,h,:].k[j,h,:] / 8
  p = softmax over i  (query axis!)
  attnw[i,h] = sum_j p[i,j,h] = sum_j exp(s[i,j,h]) / Z[j,h],  Z[j,h] = sum_i exp(s[i,j,h])
  out = einsum('ihd,ohd->io', v * attnw, Wout)

Sharding: batch 8 -> one batch per NeuronCore (data parallel), weights replicated.

v2 design (cost-model-driven):
  - Scores S^T[j,i] computed per head in fp16 as in v1 (two K=64 heads row-packed).
  - The exp+drain of each [128j, 1024i] score tile goes to ONE engine:
      * ACT tiles (48/64): scalar.activation(Exp) reads PSUM fp32 directly,
        writes fp8e4m3 g-tile to SBUF, accum_out gives Z[j] fused. No DVE copy.
      * DVE tiles (16/64): tensor_scalar Schraudolph (u8 = round(s*log2e + B))
        writes fp8 exp BITS during the mandatory PSUM drain; tensor_reduce of
        the f8 tile gives Z. Balances ACT vs DVE load.
    All g carries a uniform 2^(-6/8) downscale (ACT bias / Schraudolph B) so the
    max exp value 706 fits fp8e4m3's 448 range; the scale cancels in g/Z.
  - attnw matmul in fp8 DoubleRow: lhsT = (64/Z) broadcast via step-0 AP with
    16B-spaced pair slots, rhs = g pair tiles [128, 2, 1024]; each instruction
    consumes TWO j-tiles at 0.5 cyc/row -> 4x less PE time than v1.
    The 64x scale on 1/Z is folded out of Wout on the host (wot/64).
  - APP^T = V^T * attnw (DVE), output projection in f16 as v1.
"""
import math
import os
import numpy as np
from contextlib import ExitStack

import concourse.bass as bass
import concourse.mybir as mybir
import concourse.tile as tile
from concourse import bacc
from concourse.vector_clock import ScopedClock
from concourse.bass_utils import run_bass_kernel_spmd
import bass_rust

N_CORES = 8
B, N, C, H, D, O = 8, 1024, 256, 8, 64, 256
HD = H * D  # 512
FP32 = mybir.dt.float32
F16 = mybir.dt.float16
F8 = mybir.dt.float8e4
U8 = mybir.dt.uint8
EXP = mybir.ActivationFunctionType.Exp
ADD = mybir.AluOpType.add
MULT = mybir.AluOpType.mult
AX = mybir.AxisListType.X
DR = mybir.MatmulPerfMode.DoubleRow

# fp8 scale plumbing: g~ = exp(s/8) * 2^(-GD/8); attnw accumulated as 64*attnw;
# Wout divided by 64 on the host. Head b (f16 path) shares the same downscale
# via the common ACT bias so one bias const serves both; its Z cancels it too.
GD = 6
ACT_BIAS = -GD * math.log(2.0) / 8.0           # -0.519860
SCH_A = 1.4426950408889634                      # log2(e); arg is raw s (pre /8)
SCH_B = 56.0 - GD - 0.344                       # e4m3: 8*(bias=7) - GD - centering
SCH_A16 = 128.0 * 1.4426950408889634            # f16 bits per unit of raw s
SCH_B16 = 15360.0 - 1024.0 * GD / 8.0 - 43.5    # f16: 1024*15 - GD shift - centering
IZ_SCALE = 1.0 / 64.0                           # Z~ -> Z~/64; recip -> 64/Z~

# (head, jt) tiles drained+exp'd by DVE instead of ACT (per t): balances engines.
# head 0 rides the fp8 DoubleRow attnw path; head 1 the f16 path (fp8 DR can
# only write PSUM partitions 0-63).
DVE_TILES = {(0, 1), (1, 3), (0, 5), (1, 7)}

_MAXW = 1  # max sync waits this toolchain's walrus accepts per instruction


class _TC(tile.TileContext):
    """TileContext that splits semaphore waits one-per-instruction.

    The walrus build in this toolchain rejects any instruction carrying more
    than one sync wait ("Too many sync wait commands"), while Tile's
    add_semaphores attaches all needed waits to the consuming instruction.
    Engines execute in order, so moving excess waits onto same-engine NOPs
    emitted immediately before the instruction is semantically identical.
    """

    def _commit_instruction(self, inst, lazy_reg_writes: bool = True):
        si = inst.sync_info
        if (
            si is not None
            and si.on_wait
            and len(si.on_wait) > _MAXW
            and inst.engine != mybir.EngineType.Unassigned
        ):
            waits = list(si.on_wait)
            inst.sync_info = bass_rust.SyncInfo(
                on_wait=waits[-_MAXW:], on_update=list(si.on_update or [])
            )
            for i in range(0, len(waits) - _MAXW, _MAXW):
                nop = self.nc.engines[inst.engine].nop(nofuse=True, hint="waitsplit")
                nop.ins.sync_info = bass_rust.SyncInfo(
                    on_wait=waits[i : i + _MAXW], on_update=[]
                )
        return super()._commit_instruction(inst, lazy_reg_writes)

    def _drain_and_barrier(self, tick_clock, wait_clock):
        probe = self.nc.sync.drain()
        wait_clock.add_sem_waits(
            probe.ins, ScopedClock({None: tick_clock.global_clock})
        )
        si = probe.ins.sync_info
        waits = list(si.on_wait or []) if si is not None else []
        if len(waits) > 1:
            probe.ins.sync_info = bass_rust.SyncInfo(
                on_wait=waits[:1], on_update=list(si.on_update or [])
            )
            for i in range(1, len(waits)):
                d = self.nc.sync.drain()
                d.ins.sync_info = bass_rust.SyncInfo(
                    on_wait=waits[i : i + 1], on_update=[]
                )
        self.nc.all_engine_barrier()
        assert self.sems is not None
        popped = self.nc._tile_sem_poison_stack.pop()
        assert popped is self._sem_poison
        self.nc.clear_and_free_semaphores(list(self.sems.allocated().values()))
        self.nc.all_engine_barrier()


def _emit_body(tc, xt, wqkv, wot, out):
    nc = tc.nc
    with ExitStack() as ctx:
        wpool = ctx.enter_context(tc.tile_pool(name="w", bufs=1))
        qkvpool = ctx.enter_context(tc.tile_pool(name="qkv", bufs=1))
        gpool = ctx.enter_context(tc.tile_pool(name="g", bufs=10))
        zpool = ctx.enter_context(tc.tile_pool(name="z", bufs=2))
        izpool = ctx.enter_context(tc.tile_pool(name="iz", bufs=2))
        obpool = ctx.enter_context(tc.tile_pool(name="ob", bufs=2))

        # input loads, ordered by first use
        XT = [[None, None], [None, None]]   # [kc][ic] -> [128, 512]
        WQC = [[None, None], [None, None], [None, None]]  # [col][kc]
        WOT = []

        def load_x(kc, ic):
            t = wpool.tile([128, 512], F16, tag=f"xt{kc}{ic}", name=f"xt{kc}{ic}")
            nc.sync.dma_start(
                t[:], xt[kc * 128 : (kc + 1) * 128, ic * 512 : (ic + 1) * 512]
            )
            XT[kc][ic] = t

        def load_w(col, kc):
            w = wpool.tile([128, HD], F16, tag=f"w{col}{kc}", name=f"w{col}{kc}")
            nc.sync.dma_start(
                w[:], wqkv[kc * 128 : (kc + 1) * 128, col * HD : (col + 1) * HD]
            )
            WQC[col][kc] = w

        load_x(0, 0); load_x(1, 0); load_w(0, 0); load_w(0, 1)
        load_x(0, 1); load_x(1, 1); load_w(1, 0); load_w(1, 1)
        load_w(2, 0); load_w(2, 1)
        for kt in range(4):
            w = wpool.tile([128, O], F16, tag=f"wot{kt}", name=f"wot{kt}")
            nc.sync.dma_start(w[:], wot[kt * 128 : (kt + 1) * 128, :])
            WOT.append(w)

        actbias = wpool.tile([128, 1], FP32, tag="actbias", name="actbias")
        nc.vector.memset(actbias[:], ACT_BIAS)

        QT = [qkvpool.tile([128, N], F16, tag=f"q{m}", name=f"q{m}") for m in range(4)]
        KT = [qkvpool.tile([128, N], F16, tag=f"k{m}", name=f"k{m}") for m in range(4)]
        VT = [qkvpool.tile([128, N], F16, tag=f"v{m}", name=f"v{m}") for m in range(4)]
        APP = [qkvpool.tile([128, N], F16, tag=f"app{m}", name=f"app{m}") for m in range(4)]

        with (
            tc.tile_pool(name="sps", bufs=3, space="PSUM") as sps,
            tc.tile_pool(name="awps", bufs=1, space="PSUM") as awps,
        ):

            def project(col, m, dst):
                """dst[hd', i] = sum_c W[c, col*HD + m*128 + hd'] * xT[c, i]"""
                ps = sps.tile([128, N], FP32, tag="s")
                for ic in range(2):
                    for kc in range(2):
                        nc.tensor.matmul(
                            ps[:, ic * 512 : (ic + 1) * 512],
                            WQC[col][kc][:, m * 128 : (m + 1) * 128],
                            XT[kc][ic][:],
                            start=(kc == 0),
                            stop=(kc == 1),
                        )
                with nc.allow_low_precision(reason="f16 activations"):
                    nc.vector.tensor_copy(dst[:], ps[:])

            aw_prev = [None]  # aw psum of t-1, consumed by APP at t's jt==2

            def emit_app(tprev):
                with nc.allow_low_precision(reason="f16 activations"):
                    nc.vector.tensor_mul(APP[tprev][:], VT[tprev][:], aw_prev[0][:])

            for t in range(4):  # head pair (2t, 2t+1)
                project(0, t, QT[t])
                project(1, t, KT[t])
                project(2, t, VT[t])
                za = zpool.tile([128, 8], FP32, tag="za")
                zb = zpool.tile([128, 8], FP32, tag="zb")
                gp = {}
                for jt in range(8):
                    jsl = slice(jt * 128, (jt + 1) * 128)
                    sa = sps.tile([128, N], FP32, tag="s")
                    sb_ = sps.tile([128, N], FP32, tag="s")
                    for ic in range(2):
                        icsl = slice(ic * 512, (ic + 1) * 512)
                        # two K=64 matmuls row-packed in the PE array
                        nc.tensor.matmul(
                            sa[:, icsl], KT[t][0:64, jsl], QT[t][0:64, icsl],
                            start=True, stop=True,
                        )
                        nc.tensor.matmul(
                            sb_[:, icsl], KT[t][64:128, jsl], QT[t][64:128, icsl],
                            start=True, stop=True, tile_position=(64, 0),
                        )
                    pair, half = jt // 2, jt % 2
                    # head 0: fp8 g (DoubleRow pair tiles); head 1: f16 g
                    if half == 0:
                        gp[0, pair] = gpool.tile(
                            [128, 2, N], F8, tag="g", name=f"ga{pair}"
                        )
                    gb = gpool.tile([128, N], F16, tag="gb", name=f"gb{jt}")
                    gp[1, jt] = gb
                    for head, s_t, z_t in ((0, sa, za), (1, sb_, zb)):
                        gdst = gp[0, pair][:, half, :] if head == 0 else gb[:]
                        z_col = z_t[:, jt : jt + 1]
                        if (head, jt) in DVE_TILES:
                            a_, b_ = (SCH_A, SCH_B) if head == 0 else (SCH_A16, SCH_B16)
                            u_ = U8 if head == 0 else mybir.dt.int16
                            with nc.allow_low_precision(reason="exp bits"):
                                nc.vector.tensor_scalar(
                                    gdst.bitcast(u_), s_t[:], a_, b_, MULT, ADD
                                )
                            nc.vector.tensor_reduce(z_col, gdst, AX, ADD)
                        else:
                            nc.scalar.activation(
                                gdst, s_t[:], EXP, scale=0.125, bias=actbias[:],
                                accum_out=z_col,
                            )
                    if t > 0 and jt == 2:
                        emit_app(t - 1)

                # head a: iz strip, slot for jt at byte 16*jt; value = 64 / Z~
                # head b: izbc [128, 8, 64] f16, jt-slot broadcast materialized
                # (step-0 reads are only trusted where device-verified: DR lhsT)
                iza = izpool.tile([128, 128], F8, tag="iza", name="iza")
                izbc = izpool.tile([128, 8, 64], F16, tag="izbc", name="izbc")
                zsa = zpool.tile([128, 8], FP32, tag="zsa")
                zsb = zpool.tile([128, 8], FP32, tag="zsb")
                with nc.allow_low_precision(reason="low-precision iz"):
                    nc.vector.tensor_scalar_mul(zsa[:], za[:], IZ_SCALE)
                    izap = iza[:]
                    slots = bass.AP(
                        izap.tensor, izap.offset, [list(izap.ap[0]), [16, 8]]
                    )
                    nc.vector.reciprocal(slots, zsa[:])
                    nc.vector.tensor_scalar_mul(zsb[:], zb[:], IZ_SCALE)
                    zsbap = zsb[:]
                    zsb_bc = bass.AP(
                        zsbap.tensor, zsbap.offset,
                        [list(zsbap.ap[0]), [1, 8], [0, 64]],
                    )
                    nc.vector.reciprocal(izbc[:], zsb_bc)

                aw = awps.tile([128, N], FP32, tag="aw")
                izap = iza[:]
                for pair in range(4):
                    lhsT = bass.AP(
                        izap.tensor,
                        izap.offset + 32 * pair,
                        [list(izap.ap[0]), [16, 2], [0, 64]],
                    )
                    for ic in range(2):
                        rhs = gp[0, pair][:, :, ic * 512 : (ic + 1) * 512]
                        nc.tensor.matmul(
                            aw[0:64, ic * 512 : (ic + 1) * 512],
                            lhsT, rhs,
                            start=(pair == 0), stop=(pair == 3),
                            perf_mode=DR,
                            skip_group_check=True,
                        )
                for jt in range(8):
                    lhsT_b = izbc[:, jt, :]
                    for ic in range(2):
                        nc.tensor.matmul(
                            aw[64:128, ic * 512 : (ic + 1) * 512],
                            lhsT_b, gp[1, jt][:, ic * 512 : (ic + 1) * 512],
                            start=(jt == 0), stop=(jt == 7),
                            tile_position=(0, 64),
                            skip_group_check=True,
                        )
                aw_prev[0] = aw
            emit_app(3)

            for it in range(8):
                itsl = slice(it * 128, (it + 1) * 128)
                po = sps.tile([128, O], FP32, tag="s")
                for kt in range(4):
                    nc.tensor.matmul(
                        po[:], APP[kt][:, itsl], WOT[kt][:],
                        start=(kt == 0), stop=(kt == 3),
                    )
                ob = obpool.tile([128, O], FP32, tag="ob")
                nc.vector.tensor_copy(ob[:], po[:])
                nc.sync.dma_start(out[itsl, :], ob[:])


def build_nc(loop=0, use_bacc=False):
    cls = bacc.Bacc if use_bacc else bass.Bass
    nc = cls("TRN2", target_bir_lowering=False, debug=False, num_devices=N_CORES)
    xt = nc.declare_dram_parameter("xt", [C, N], F16, isOutput=False)
    wqkv = nc.declare_dram_parameter("wqkv", [C, 3 * HD], F16, isOutput=False)
    wot = nc.declare_dram_parameter("wot", [HD, O], F16, isOutput=False)
    out = nc.declare_dram_parameter("out", [N, O], FP32, isOutput=True)
    with _TC(nc, num_cores=N_CORES) as tc:
        if loop:
            with tc.For_i(0, loop, 1):
                _emit_body(tc, xt.ap(), wqkv.ap(), wot.ap(), out.ap())
        else:
            _emit_body(tc, xt.ap(), wqkv.ap(), wot.ap(), out.ap())
    return nc


def make_in_maps(features, weight_q, weight_k, weight_v, weight_out):
    wqkv = np.ascontiguousarray(
        np.concatenate(
            [
                weight_q.reshape(C, HD),
                weight_k.reshape(C, HD),
                weight_v.reshape(C, HD),
            ],
            axis=1,
        ),
        dtype=np.float16,
    )
    # attnw arrives scaled by 64 (iz = 64/Z); fold the 1/64 into Wout.
    wot = np.ascontiguousarray(
        weight_out.reshape(O, HD).T * (1.0 / 64.0), dtype=np.float16
    )
    in_maps = []
    for b in range(B):
        xt = np.ascontiguousarray(features[b].T, dtype=np.float16)
        in_maps.append({"xt": xt, "wqkv": wqkv, "wot": wot})
    return in_maps


_CACHED_NC = None


def kernel(features, weight_q, weight_k, weight_v, weight_out):
    global _CACHED_NC
    if _CACHED_NC is None:
        _CACHED_NC = build_nc(loop=0)
    in_maps = make_in_maps(
        np.asarray(features, np.float32),
        np.asarray(weight_q, np.float32),
        np.asarray(weight_k, np.float32),
        np.asarray(weight_v, np.float32),
        np.asarray(weight_out, np.float32),
    )
    res = run_bass_kernel_spmd(_CACHED_NC, in_maps, list(range(N_CORES)))
    return np.stack([res.results[b]["out"] for b in range(B)], axis=0)


if __name__ == "__main__":
    rng = np.random.default_rng(0)
    feats = rng.standard_normal((B, N, C)).astype(np.float32)
    wq = rng.standard_normal((C, H, D)).astype(np.float32) * 0.05
    wk = rng.standard_normal((C, H, D)).astype(np.float32) * 0.05
    wv = rng.standard_normal((C, H, D)).astype(np.float32) * 0.05
    wo = rng.standard_normal((O, H, D)).astype(np.float32) * 0.05
    o = kernel(feats, wq, wk, wv, wo)
    print("kernel ran, out shape", o.shape, "finite:", np.isfinite(o).all())


# revision 18
# speedup vs baseline: 1.1451x; 1.1451x over previous
"""Trainium2 Bass kernel for nn_MultiHeadAttention_62371515073076 (v2).

Math (per batch b, faithful to the reference's quirky softmax over the QUERY axis):
  q/k/v = einsum('nc,chd->nhd', x, W{q,k,v})
  s[i,j,h] = q[i,h,:].k[j,h,:] / 8
  p = softmax over i  (query axis!)
  attnw[i,h] = sum_j p[i,j,h] = sum_j exp(s[i,j,h]) / Z[j,h],  Z[j,h] = sum_i exp(s[i,j,h])
  out = einsum('ihd,ohd->io', v * attnw, Wout)

Sharding: batch 8 -> one batch per NeuronCore (data parallel), weights replicated.

v2 design (cost-model-driven):
  - Scores S^T[j,i] computed per head in fp16 as in v1 (two K=64 heads row-packed).
  - The exp+drain of each [128j, 1024i] score tile goes to ONE engine:
      * ACT tiles (48/64): scalar.activation(Exp) reads PSUM fp32 directly,
        writes fp8e4m3 g-tile to SBUF, accum_out gives Z[j] fused. No DVE copy.
      * DVE tiles (16/64): tensor_scalar Schraudolph (u8 = round(s*log2e + B))
        writes fp8 exp BITS during the mandatory PSUM drain; tensor_reduce of
        the f8 tile gives Z. Balances ACT vs DVE load.
    All g carries a uniform 2^(-6/8) downscale (ACT bias / Schraudolph B) so the
    max exp value 706 fits fp8e4m3's 448 range; the scale cancels in g/Z.
  - attnw matmul in fp8 DoubleRow: lhsT = (64/Z) broadcast via step-0 AP with
    16B-spaced pair slots, rhs = g pair tiles [128, 2, 1024]; each instruction
    consumes TWO j-tiles at 0.5 cyc/row -> 4x less PE time than v1.
    The 64x scale on 1/Z is folded out of Wout on the host (wot/64).
  - APP^T = V^T * attnw (DVE), output projection in f16 as v1.
"""
import math
import os
import numpy as np
from contextlib import ExitStack

import concourse.bass as bass
import concourse.mybir as mybir
import concourse.tile as tile
from concourse import bacc
from concourse.vector_clock import ScopedClock
from concourse.bass_utils import run_bass_kernel_spmd
import bass_rust

N_CORES = 8
B, N, C, H, D, O = 8, 1024, 256, 8, 64, 256
HD = H * D  # 512
FP32 = mybir.dt.float32
F16 = mybir.dt.float16
F8 = mybir.dt.float8e4
U8 = mybir.dt.uint8
EXP = mybir.ActivationFunctionType.Exp
ADD = mybir.AluOpType.add
MULT = mybir.AluOpType.mult
AX = mybir.AxisListType.X
DR = mybir.MatmulPerfMode.DoubleRow

# fp8 scale plumbing: g~ = exp(s/8) * 2^(-GD/8); attnw accumulated as 64*attnw;
# Wout divided by 64 on the host. Head b (f16 path) shares the same downscale
# via the common ACT bias so one bias const serves both; its Z cancels it too.
GD = 6
ACT_BIAS = -GD * math.log(2.0) / 8.0           # -0.519860
SCH_A = 1.4426950408889634                      # log2(e); arg is raw s (pre /8)
SCH_B = 56.0 - GD - 0.344                       # e4m3: 8*(bias=7) - GD - centering
SCH_A16 = 128.0 * 1.4426950408889634            # f16 bits per unit of raw s
SCH_B16 = 15360.0 - 1024.0 * GD / 8.0 - 43.5    # f16: 1024*15 - GD shift - centering
IZ_SCALE = 1.0 / 64.0                           # Z~ -> Z~/64; recip -> 64/Z~

# (head, jt) tiles drained+exp'd by DVE instead of ACT (per t): balances engines.
# head 0 rides the fp8 DoubleRow attnw path; head 1 the f16 path (fp8 DR can
# only write PSUM partitions 0-63).
DVE_TILES = {(0, 1), (1, 2), (0, 4), (1, 5)}

_MAXW = 1  # max sync waits this toolchain's walrus accepts per instruction


class _TC(tile.TileContext):
    """TileContext that splits semaphore waits one-per-instruction.

    The walrus build in this toolchain rejects any instruction carrying more
    than one sync wait ("Too many sync wait commands"), while Tile's
    add_semaphores attaches all needed waits to the consuming instruction.
    Engines execute in order, so moving excess waits onto same-engine NOPs
    emitted immediately before the instruction is semantically identical.
    """

    def _commit_instruction(self, inst, lazy_reg_writes: bool = True):
        si = inst.sync_info
        if (
            si is not None
            and si.on_wait
            and len(si.on_wait) > _MAXW
            and inst.engine != mybir.EngineType.Unassigned
        ):
            waits = list(si.on_wait)
            inst.sync_info = bass_rust.SyncInfo(
                on_wait=waits[-_MAXW:], on_update=list(si.on_update or [])
            )
            for i in range(0, len(waits) - _MAXW, _MAXW):
                nop = self.nc.engines[inst.engine].nop(nofuse=True, hint="waitsplit")
                nop.ins.sync_info = bass_rust.SyncInfo(
                    on_wait=waits[i : i + _MAXW], on_update=[]
                )
        return super()._commit_instruction(inst, lazy_reg_writes)

    def _drain_and_barrier(self, tick_clock, wait_clock):
        probe = self.nc.sync.drain()
        wait_clock.add_sem_waits(
            probe.ins, ScopedClock({None: tick_clock.global_clock})
        )
        si = probe.ins.sync_info
        waits = list(si.on_wait or []) if si is not None else []
        if len(waits) > 1:
            probe.ins.sync_info = bass_rust.SyncInfo(
                on_wait=waits[:1], on_update=list(si.on_update or [])
            )
            for i in range(1, len(waits)):
                d = self.nc.sync.drain()
                d.ins.sync_info = bass_rust.SyncInfo(
                    on_wait=waits[i : i + 1], on_update=[]
                )
        self.nc.all_engine_barrier()
        assert self.sems is not None
        popped = self.nc._tile_sem_poison_stack.pop()
        assert popped is self._sem_poison
        self.nc.clear_and_free_semaphores(list(self.sems.allocated().values()))
        self.nc.all_engine_barrier()


def _emit_body(tc, xt, wqkv, wot, out):
    nc = tc.nc
    with ExitStack() as ctx:
        wpool = ctx.enter_context(tc.tile_pool(name="w", bufs=1))
        qkvpool = ctx.enter_context(tc.tile_pool(name="qkv", bufs=1))
        gpool = ctx.enter_context(tc.tile_pool(name="g", bufs=10))
        zpool = ctx.enter_context(tc.tile_pool(name="z", bufs=2))
        izpool = ctx.enter_context(tc.tile_pool(name="iz", bufs=2))
        obpool = ctx.enter_context(tc.tile_pool(name="ob", bufs=2))

        # input loads, ordered by first use
        XT = [[None, None], [None, None]]   # [kc][ic] -> [128, 512]
        WQC = [[None, None], [None, None], [None, None]]  # [col][kc]
        WOT = []

        def load_x(kc, ic):
            t = wpool.tile([128, 512], F16, tag=f"xt{kc}{ic}", name=f"xt{kc}{ic}")
            nc.sync.dma_start(
                t[:], xt[kc * 128 : (kc + 1) * 128, ic * 512 : (ic + 1) * 512]
            )
            XT[kc][ic] = t

        def load_w(col, kc):
            w = wpool.tile([128, HD], F16, tag=f"w{col}{kc}", name=f"w{col}{kc}")
            nc.sync.dma_start(
                w[:], wqkv[kc * 128 : (kc + 1) * 128, col * HD : (col + 1) * HD]
            )
            WQC[col][kc] = w

        load_x(0, 0); load_x(1, 0); load_w(0, 0); load_w(0, 1)
        load_x(0, 1); load_x(1, 1); load_w(1, 0); load_w(1, 1)
        load_w(2, 0); load_w(2, 1)
        for kt in range(4):
            w = wpool.tile([128, O], F16, tag=f"wot{kt}", name=f"wot{kt}")
            nc.sync.dma_start(w[:], wot[kt * 128 : (kt + 1) * 128, :])
            WOT.append(w)

        actbias = wpool.tile([128, 1], FP32, tag="actbias", name="actbias")
        nc.vector.memset(actbias[:], ACT_BIAS)

        QT = [qkvpool.tile([128, N], F16, tag=f"q{m}", name=f"q{m}") for m in range(4)]
        KT = [qkvpool.tile([128, N], F16, tag=f"k{m}", name=f"k{m}") for m in range(4)]
        VT = [qkvpool.tile([128, N], F16, tag=f"v{m}", name=f"v{m}") for m in range(4)]
        APP = [qkvpool.tile([128, N], F16, tag=f"app{m}", name=f"app{m}") for m in range(4)]

        with (
            tc.tile_pool(name="sps", bufs=2, space="PSUM") as sps,
            tc.tile_pool(name="pps", bufs=1, space="PSUM") as pps,
            tc.tile_pool(name="awps", bufs=1, space="PSUM") as awps,
        ):

            def project(col, m, dst, pool=None):
                """dst[hd', i] = sum_c W[c, col*HD + m*128 + hd'] * xT[c, i]"""
                ps = (pool or pps).tile([128, N], FP32, tag="s")
                for kc in range(2):
                    for ic in range(2):
                        nc.tensor.matmul(
                            ps[:, ic * 512 : (ic + 1) * 512],
                            WQC[col][kc][:, m * 128 : (m + 1) * 128],
                            XT[kc][ic][:],
                            start=(kc == 0),
                            stop=(kc == 1),
                        )
                with nc.allow_low_precision(reason="f16 activations"):
                    nc.vector.tensor_copy(dst[:], ps[:])

            aw_prev = [None]  # aw psum of t-1, consumed by APP at t's jt==4

            def emit_app(tprev):
                with nc.allow_low_precision(reason="f16 activations"):
                    nc.vector.tensor_mul(APP[tprev][:], VT[tprev][:], aw_prev[0][:])

            project(0, 0, QT[0], pool=sps)
            project(1, 0, KT[0], pool=sps)
            project(2, 0, VT[0], pool=pps)
            for t in range(4):  # head pair (2t, 2t+1)
                za = zpool.tile([128, 8], FP32, tag="za")
                zb = zpool.tile([128, 8], FP32, tag="zb")
                zsa = zpool.tile([128, 8], FP32, tag="zsa")
                # padded: walrus bounds-checks step-0 bcast reads as advancing
                zsb = zpool.tile([128, 72], FP32, tag="zsb")
                iza = izpool.tile([128, 128], F8, tag="iza", name="iza")
                izbc = izpool.tile([128, 8, 64], F16, tag="izbc", name="izbc")
                aw = awps.tile([128, N], FP32, tag="aw")
                gp = {}
                for jt in range(8):
                    jsl = slice(jt * 128, (jt + 1) * 128)
                    sa = sps.tile([128, N], FP32, tag="s")
                    sb_ = sps.tile([128, N], FP32, tag="s")
                    for ic in range(2):
                        icsl = slice(ic * 512, (ic + 1) * 512)
                        # K=64 head-a matmuls (PE rows 0-63); lhsT reused over ic
                        nc.tensor.matmul(
                            sa[:, icsl], KT[t][0:64, jsl], QT[t][0:64, icsl],
                            start=True, stop=True,
                        )
                    for ic in range(2):
                        icsl = slice(ic * 512, (ic + 1) * 512)
                        nc.tensor.matmul(
                            sb_[:, icsl], KT[t][64:128, jsl], QT[t][64:128, icsl],
                            start=True, stop=True, tile_position=(64, 0),
                        )
                    pair, half = jt // 2, jt % 2
                    # head 0: fp8 g (DoubleRow pair tiles); head 1: f16 g
                    if half == 0:
                        gp[0, pair] = gpool.tile(
                            [128, 2, N], F8, tag="g", name=f"ga{pair}"
                        )
                    gb = gpool.tile([128, N], F16, tag="gb", name=f"gb{jt}")
                    gp[1, jt] = gb
                    for head, s_t, z_t in ((0, sa, za), (1, sb_, zb)):
                        gdst = gp[0, pair][:, half, :] if head == 0 else gb[:]
                        z_col = z_t[:, jt : jt + 1]
                        if (head, jt) in DVE_TILES:
                            a_, b_ = (SCH_A, SCH_B) if head == 0 else (SCH_A16, SCH_B16)
                            u_ = U8 if head == 0 else mybir.dt.int16
                            with nc.allow_low_precision(reason="exp bits"):
                                nc.vector.tensor_scalar(
                                    gdst.bitcast(u_), s_t[:], a_, b_, MULT, ADD
                                )
                            nc.vector.tensor_reduce(z_col, gdst, AX, ADD)
                        else:
                            nc.scalar.activation(
                                gdst, s_t[:], EXP, scale=0.125, bias=actbias[:],
                                accum_out=z_col,
                            )
                    if t > 0 and jt == 2:
                        emit_app(t - 1)
                    if t < 3 and jt in (4, 5, 6):
                        col = jt - 4
                        dst = (QT, KT, VT)[col][t + 1]
                        project(col, t + 1, dst)
                    if half == 1:
                        # pair complete: iz prep + attnw matmul steps for pair
                        csl = slice(2 * pair, 2 * pair + 2)
                        with nc.allow_low_precision(reason="low-precision iz"):
                            nc.vector.tensor_scalar_mul(zsa[:, csl], za[:, csl], IZ_SCALE)
                            izap = iza[:]
                            slots = bass.AP(
                                izap.tensor, izap.offset + 32 * pair,
                                [list(izap.ap[0]), [16, 2]],
                            )
                            nc.vector.reciprocal(slots, zsa[:, csl])
                            nc.vector.tensor_scalar_mul(zsb[:, csl], zb[:, csl], IZ_SCALE)
                            zsbap = zsb[:]
                            zsb_bc = bass.AP(
                                zsbap.tensor, zsbap.offset + 2 * pair,
                                [list(zsbap.ap[0]), [1, 2], [0, 64]],
                            )
                            nc.vector.reciprocal(izbc[:, csl, :], zsb_bc)
                        lhsT = bass.AP(
                            izap.tensor, izap.offset + 32 * pair,
                            [list(izap.ap[0]), [16, 2], [0, 64]],
                        )
                        for ic in range(2):
                            rhs = gp[0, pair][:, :, ic * 512 : (ic + 1) * 512]
                            nc.tensor.matmul(
                                aw[0:64, ic * 512 : (ic + 1) * 512],
                                lhsT, rhs,
                                start=(pair == 0), stop=(pair == 3),
                                perf_mode=DR,
                                skip_group_check=True,
                            )
                        for jb in (2 * pair, 2 * pair + 1):
                            for ic in range(2):
                                nc.tensor.matmul(
                                    aw[64:128, ic * 512 : (ic + 1) * 512],
                                    izbc[:, jb, :],
                                    gp[1, jb][:, ic * 512 : (ic + 1) * 512],
                                    start=(jb == 0),
                                    stop=(jb == 7),
                                    tile_position=(0, 64),
                                    skip_group_check=True,
                                )

                aw_prev[0] = aw
            emit_app(3)

            for it in range(8):
                itsl = slice(it * 128, (it + 1) * 128)
                po = sps.tile([128, O], FP32, tag="s")
                for kt in range(4):
                    nc.tensor.matmul(
                        po[:], APP[kt][:, itsl], WOT[kt][:],
                        start=(kt == 0), stop=(kt == 3),
                    )
                ob = obpool.tile([128, O], FP32, tag="ob")
                nc.scalar.copy(ob[:], po[:])
                nc.scalar.dma_start(out[itsl, :], ob[:])


def build_nc(loop=0, use_bacc=False, unroll=1):
    cls = bacc.Bacc if use_bacc else bass.Bass
    nc = cls("TRN2", target_bir_lowering=False, debug=False, num_devices=N_CORES)
    xt = nc.declare_dram_parameter("xt", [C, N], F16, isOutput=False)
    wqkv = nc.declare_dram_parameter("wqkv", [C, 3 * HD], F16, isOutput=False)
    wot = nc.declare_dram_parameter("wot", [HD, O], F16, isOutput=False)
    out = nc.declare_dram_parameter("out", [N, O], FP32, isOutput=True)
    with _TC(nc, num_cores=N_CORES) as tc:
        if loop:
            with tc.For_i(0, loop, 1):
                _emit_body(tc, xt.ap(), wqkv.ap(), wot.ap(), out.ap())
        else:
            for _ in range(unroll):
                _emit_body(tc, xt.ap(), wqkv.ap(), wot.ap(), out.ap())
    return nc


def make_in_maps(features, weight_q, weight_k, weight_v, weight_out):
    wqkv = np.ascontiguousarray(
        np.concatenate(
            [
                weight_q.reshape(C, HD),
                weight_k.reshape(C, HD),
                weight_v.reshape(C, HD),
            ],
            axis=1,
        ),
        dtype=np.float16,
    )
    # attnw arrives scaled by 64 (iz = 64/Z); fold the 1/64 into Wout.
    wot = np.ascontiguousarray(
        weight_out.reshape(O, HD).T * (1.0 / 64.0), dtype=np.float16
    )
    in_maps = []
    for b in range(B):
        xt = np.ascontiguousarray(features[b].T, dtype=np.float16)
        in_maps.append({"xt": xt, "wqkv": wqkv, "wot": wot})
    return in_maps


_CACHED_NC = None


def kernel(features, weight_q, weight_k, weight_v, weight_out):
    global _CACHED_NC
    if _CACHED_NC is None:
        _CACHED_NC = build_nc(loop=0)
    in_maps = make_in_maps(
        np.asarray(features, np.float32),
        np.asarray(weight_q, np.float32),
        np.asarray(weight_k, np.float32),
        np.asarray(weight_v, np.float32),
        np.asarray(weight_out, np.float32),
    )
    res = run_bass_kernel_spmd(_CACHED_NC, in_maps, list(range(N_CORES)))
    return np.stack([res.results[b]["out"] for b in range(B)], axis=0)


if __name__ == "__main__":
    rng = np.random.default_rng(0)
    feats = rng.standard_normal((B, N, C)).astype(np.float32)
    wq = rng.standard_normal((C, H, D)).astype(np.float32) * 0.05
    wk = rng.standard_normal((C, H, D)).astype(np.float32) * 0.05
    wv = rng.standard_normal((C, H, D)).astype(np.float32) * 0.05
    wo = rng.standard_normal((O, H, D)).astype(np.float32) * 0.05
    o = kernel(feats, wq, wk, wv, wo)
    print("kernel ran, out shape", o.shape, "finite:", np.isfinite(o).all())


# revision 19
# speedup vs baseline: 1.3462x; 1.1756x over previous
"""Trainium2 Bass kernel for nn_MultiHeadAttention_62371515073076 (v2).

Math (per batch b, faithful to the reference's quirky softmax over the QUERY axis):
  q/k/v = einsum('nc,chd->nhd', x, W{q,k,v})
  s[i,j,h] = q[i,h,:].k[j,h,:] / 8
  p = softmax over i  (query axis!)
  attnw[i,h] = sum_j p[i,j,h] = sum_j exp(s[i,j,h]) / Z[j,h],  Z[j,h] = sum_i exp(s[i,j,h])
  out = einsum('ihd,ohd->io', v * attnw, Wout)

Sharding: batch 8 -> one batch per NeuronCore (data parallel), weights replicated.

v2 design (cost-model-driven):
  - Scores S^T[j,i] computed per head in fp16 as in v1 (two K=64 heads row-packed).
  - The exp+drain of each [128j, 1024i] score tile goes to ONE engine:
      * ACT tiles (48/64): scalar.activation(Exp) reads PSUM fp32 directly,
        writes fp8e4m3 g-tile to SBUF, accum_out gives Z[j] fused. No DVE copy.
      * DVE tiles (16/64): tensor_scalar Schraudolph (u8 = round(s*log2e + B))
        writes fp8 exp BITS during the mandatory PSUM drain; tensor_reduce of
        the f8 tile gives Z. Balances ACT vs DVE load.
    All g carries a uniform 2^(-6/8) downscale (ACT bias / Schraudolph B) so the
    max exp value 706 fits fp8e4m3's 448 range; the scale cancels in g/Z.
  - attnw matmul in fp8 DoubleRow: lhsT = (64/Z) broadcast via step-0 AP with
    16B-spaced pair slots, rhs = g pair tiles [128, 2, 1024]; each instruction
    consumes TWO j-tiles at 0.5 cyc/row -> 4x less PE time than v1.
    The 64x scale on 1/Z is folded out of Wout on the host (wot/64).
  - APP^T = V^T * attnw (DVE), output projection in f16 as v1.
"""
import math
import os
import numpy as np
from contextlib import ExitStack

import concourse.bass as bass
import concourse.mybir as mybir
import concourse.tile as tile
from concourse import bacc
from concourse.vector_clock import ScopedClock
from concourse.bass_utils import run_bass_kernel_spmd
import bass_rust

N_CORES = 8
B, N, C, H, D, O = 8, 1024, 256, 8, 64, 256
HD = H * D  # 512
FP32 = mybir.dt.float32
F16 = mybir.dt.float16
F8 = mybir.dt.float8e4
U8 = mybir.dt.uint8
EXP = mybir.ActivationFunctionType.Exp
ADD = mybir.AluOpType.add
MULT = mybir.AluOpType.mult
AX = mybir.AxisListType.X
DR = mybir.MatmulPerfMode.DoubleRow

# fp8 scale plumbing: g~ = exp(s/8) * 2^(-GD/8); attnw accumulated as 64*attnw;
# Wout divided by 64 on the host. Head b (f16 path) shares the same downscale
# via the common ACT bias so one bias const serves both; its Z cancels it too.
GD = 6
ACT_BIAS = -GD * math.log(2.0) / 8.0           # -0.519860
SCH_A = 1.4426950408889634                      # log2(e); arg is raw s (pre /8)
SCH_B = 56.0 - GD - 0.344                       # e4m3: 8*(bias=7) - GD - centering
SCH_A16 = 128.0 * 1.4426950408889634            # f16 bits per unit of raw s
SCH_B16 = 15360.0 - 1024.0 * GD / 8.0 - 43.5    # f16: 1024*15 - GD shift - centering
IZ_SCALE = 1.0 / 64.0                           # Z~ -> Z~/64; recip -> 64/Z~

# (head, jt) tiles drained+exp'd by DVE instead of ACT (per t): balances engines.
# head 0 rides the fp8 DoubleRow attnw path; head 1 the f16 path (fp8 DR can
# only write PSUM partitions 0-63).
DVE_TILES = {(0, 1), (1, 2), (0, 4), (1, 5)}

_MAXW = 1  # max sync waits this toolchain's walrus accepts per instruction


class _TC(tile.TileContext):
    """TileContext that splits semaphore waits one-per-instruction.

    The walrus build in this toolchain rejects any instruction carrying more
    than one sync wait ("Too many sync wait commands"), while Tile's
    add_semaphores attaches all needed waits to the consuming instruction.
    Engines execute in order, so moving excess waits onto same-engine NOPs
    emitted immediately before the instruction is semantically identical.
    """

    def _commit_instruction(self, inst, lazy_reg_writes: bool = True):
        si = inst.sync_info
        if (
            si is not None
            and si.on_wait
            and len(si.on_wait) > _MAXW
            and inst.engine != mybir.EngineType.Unassigned
        ):
            waits = list(si.on_wait)
            inst.sync_info = bass_rust.SyncInfo(
                on_wait=waits[-_MAXW:], on_update=list(si.on_update or [])
            )
            for i in range(0, len(waits) - _MAXW, _MAXW):
                nop = self.nc.engines[inst.engine].nop(nofuse=True, hint="waitsplit")
                nop.ins.sync_info = bass_rust.SyncInfo(
                    on_wait=waits[i : i + _MAXW], on_update=[]
                )
        return super()._commit_instruction(inst, lazy_reg_writes)

    def _drain_and_barrier(self, tick_clock, wait_clock):
        probe = self.nc.sync.drain()
        wait_clock.add_sem_waits(
            probe.ins, ScopedClock({None: tick_clock.global_clock})
        )
        si = probe.ins.sync_info
        waits = list(si.on_wait or []) if si is not None else []
        if len(waits) > 1:
            probe.ins.sync_info = bass_rust.SyncInfo(
                on_wait=waits[:1], on_update=list(si.on_update or [])
            )
            for i in range(1, len(waits)):
                d = self.nc.sync.drain()
                d.ins.sync_info = bass_rust.SyncInfo(
                    on_wait=waits[i : i + 1], on_update=[]
                )
        self.nc.all_engine_barrier()
        assert self.sems is not None
        popped = self.nc._tile_sem_poison_stack.pop()
        assert popped is self._sem_poison
        self.nc.clear_and_free_semaphores(list(self.sems.allocated().values()))
        self.nc.all_engine_barrier()


def _emit_body(tc, xt, wqkv, wot, out):
    nc = tc.nc
    with ExitStack() as ctx:
        wpool = ctx.enter_context(tc.tile_pool(name="w", bufs=1))
        qkvpool = ctx.enter_context(tc.tile_pool(name="qkv", bufs=1))
        gpool = ctx.enter_context(tc.tile_pool(name="g", bufs=10))
        zpool = ctx.enter_context(tc.tile_pool(name="z", bufs=2))
        izpool = ctx.enter_context(tc.tile_pool(name="iz", bufs=2))
        obpool = ctx.enter_context(tc.tile_pool(name="ob", bufs=2))

        # input loads, ordered by first use
        XT = [[None, None], [None, None]]   # [kc][ic] -> [128, 512]
        WQC = [[None, None], [None, None], [None, None]]  # [col][kc]
        WOT = []

        def load_x(kc, ic):
            t = wpool.tile([128, 512], F16, tag=f"xt{kc}{ic}", name=f"xt{kc}{ic}")
            nc.sync.dma_start(
                t[:], xt[kc * 128 : (kc + 1) * 128, ic * 512 : (ic + 1) * 512]
            )
            XT[kc][ic] = t

        def load_w(col, kc):
            w = wpool.tile([128, HD], F16, tag=f"w{col}{kc}", name=f"w{col}{kc}")
            nc.sync.dma_start(
                w[:], wqkv[kc * 128 : (kc + 1) * 128, col * HD : (col + 1) * HD]
            )
            WQC[col][kc] = w

        load_x(0, 0); load_x(1, 0); load_w(0, 0); load_w(0, 1)
        load_x(0, 1); load_x(1, 1); load_w(1, 0); load_w(1, 1)
        load_w(2, 0); load_w(2, 1)
        for kt in range(4):
            w = wpool.tile([128, O], F16, tag=f"wot{kt}", name=f"wot{kt}")
            nc.sync.dma_start(w[:], wot[kt * 128 : (kt + 1) * 128, :])
            WOT.append(w)

        actbias = wpool.tile([128, 1], FP32, tag="actbias", name="actbias")
        nc.vector.memset(actbias[:], ACT_BIAS)
        ones64 = wpool.tile([1, 64], F16, tag="ones64", name="ones64")
        with nc.allow_low_precision(reason="const"):
            nc.vector.memset(ones64[:], 1.0)

        QT = [qkvpool.tile([128, N], F16, tag=f"q{m}", name=f"q{m}") for m in range(4)]
        KT = [qkvpool.tile([128, N], F16, tag=f"k{m}", name=f"k{m}") for m in range(4)]
        VT = [qkvpool.tile([128, N], F16, tag=f"v{m}", name=f"v{m}") for m in range(4)]
        APP = [qkvpool.tile([128, N], F16, tag=f"app{m}", name=f"app{m}") for m in range(4)]

        with (
            tc.tile_pool(name="sps", bufs=2, space="PSUM") as sps,
            tc.tile_pool(name="pps", bufs=1, space="PSUM") as pps,
            tc.tile_pool(name="awps", bufs=1, space="PSUM") as awps,
        ):

            def project(col, m, dst, pool=None):
                """dst[hd', i] = sum_c W[c, col*HD + m*128 + hd'] * xT[c, i]"""
                ps = (pool or pps).tile([128, N], FP32, tag="s")
                for kc in range(2):
                    for ic in range(2):
                        nc.tensor.matmul(
                            ps[:, ic * 512 : (ic + 1) * 512],
                            WQC[col][kc][:, m * 128 : (m + 1) * 128],
                            XT[kc][ic][:],
                            start=(kc == 0),
                            stop=(kc == 1),
                        )
                with nc.allow_low_precision(reason="f16 activations"):
                    nc.vector.tensor_copy(dst[:], ps[:])

            aw_prev = [None]  # aw psum of t-1, consumed by APP at t's jt==4

            def emit_app(tprev):
                with nc.allow_low_precision(reason="f16 activations"):
                    nc.vector.tensor_mul(APP[tprev][:], VT[tprev][:], aw_prev[0][:])

            project(0, 0, QT[0], pool=sps)
            project(1, 0, KT[0], pool=sps)
            project(2, 0, VT[0], pool=pps)
            for t in range(4):  # head pair (2t, 2t+1)
                za = zpool.tile([128, 8], FP32, tag="za")
                zb = zpool.tile([128, 8], FP32, tag="zb")
                zsa = zpool.tile([128, 8], FP32, tag="zsa")
                # padded: walrus bounds-checks step-0 bcast reads as advancing
                zsb = zpool.tile([128, 72], FP32, tag="zsb")
                iza = izpool.tile([128, 128], F8, tag="iza", name="iza")
                izb = izpool.tile([128, 128], F8, tag="izb", name="izb")
                aw = awps.tile([128, N], FP32, tag="aw")
                gp = {}
                for jt in range(8):
                    jsl = slice(jt * 128, (jt + 1) * 128)
                    sa = sps.tile([128, N], FP32, tag="s")
                    sb_ = sps.tile([128, N], FP32, tag="s")
                    for ic in range(2):
                        icsl = slice(ic * 512, (ic + 1) * 512)
                        # K=64 head-a matmuls (PE rows 0-63); lhsT reused over ic
                        nc.tensor.matmul(
                            sa[:, icsl], KT[t][0:64, jsl], QT[t][0:64, icsl],
                            start=True, stop=True,
                        )
                    for ic in range(2):
                        icsl = slice(ic * 512, (ic + 1) * 512)
                        nc.tensor.matmul(
                            sb_[:, icsl], KT[t][64:128, jsl], QT[t][64:128, icsl],
                            start=True, stop=True, tile_position=(64, 0),
                        )
                    pair, half = jt // 2, jt % 2
                    # both heads: fp8 g pair tiles (DoubleRow operands)
                    if half == 0:
                        gp[0, pair] = gpool.tile(
                            [128, 2, N], F8, tag="g", name=f"ga{pair}"
                        )
                        gp[1, pair] = gpool.tile(
                            [128, 2, N], F8, tag="g", name=f"gb{pair}"
                        )
                    for head, s_t, z_t in ((0, sa, za), (1, sb_, zb)):
                        gdst = gp[head, pair][:, half, :]
                        z_col = z_t[:, jt : jt + 1]
                        if (head, jt) in DVE_TILES:
                            with nc.allow_low_precision(reason="exp bits"):
                                nc.vector.tensor_scalar(
                                    gdst.bitcast(U8), s_t[:], SCH_A, SCH_B, MULT, ADD
                                )
                            nc.vector.tensor_reduce(z_col, gdst, AX, ADD)
                        else:
                            nc.scalar.activation(
                                gdst, s_t[:], EXP, scale=0.125, bias=actbias[:],
                                accum_out=z_col,
                            )
                    if t > 0 and jt == 2:
                        emit_app(t - 1)
                    if t < 3 and jt in (4, 5, 6):
                        col = jt - 4
                        dst = (QT, KT, VT)[col][t + 1]
                        project(col, t + 1, dst)
                    if half == 1:
                        # pair complete: iz prep + head-a attnw DR steps
                        csl = slice(2 * pair, 2 * pair + 2)
                        with nc.allow_low_precision(reason="low-precision iz"):
                            nc.vector.tensor_scalar_mul(zsa[:, csl], za[:, csl], IZ_SCALE)
                            izap = iza[:]
                            slots = bass.AP(
                                izap.tensor, izap.offset + 32 * pair,
                                [list(izap.ap[0]), [16, 2]],
                            )
                            nc.vector.reciprocal(slots, zsa[:, csl])
                            nc.vector.tensor_scalar_mul(zsb[:, csl], zb[:, csl], IZ_SCALE)
                            izbp = izb[:]
                            slots_b = bass.AP(
                                izbp.tensor, izbp.offset + 32 * pair,
                                [list(izbp.ap[0]), [16, 2]],
                            )
                            nc.vector.reciprocal(slots_b, zsb[:, csl])
                        lhsT = bass.AP(
                            izap.tensor, izap.offset + 32 * pair,
                            [list(izap.ap[0]), [16, 2], [0, 64]],
                        )
                        for ic in range(2):
                            rhs = gp[0, pair][:, :, ic * 512 : (ic + 1) * 512]
                            nc.tensor.matmul(
                                aw[0:64, ic * 512 : (ic + 1) * 512],
                                lhsT, rhs,
                                start=(pair == 0), stop=(pair == 3),
                                perf_mode=DR,
                                skip_group_check=True,
                            )

                # head-b attnw: M=1 DR rows into a pps-slot psum row, then
                # PE ones-broadcast into aw partitions 64-127
                awrow_t = pps.tile([128, N], FP32, tag="s")
                awrow = awrow_t[0:1, :]
                izbp = izb[:]
                for pair in range(4):
                    lhsT_b = bass.AP(
                        izbp.tensor, izbp.offset + 32 * pair,
                        [list(izbp.ap[0]), [16, 2], [0, 1]],
                    )
                    for ic in range(2):
                        nc.tensor.matmul(
                            awrow[:, ic * 512 : (ic + 1) * 512],
                            lhsT_b, gp[1, pair][:, :, ic * 512 : (ic + 1) * 512],
                            start=(pair == 0), stop=(pair == 3),
                            perf_mode=DR,
                            skip_group_check=True,
                        )
                rowsb = zpool.tile([1, N], F16, tag="rowsb", name="rowsb")
                with nc.allow_low_precision(reason="f16 attnw row"):
                    nc.vector.tensor_copy(rowsb[:], awrow)
                for ic in range(2):
                    nc.tensor.matmul(
                        aw[64:128, ic * 512 : (ic + 1) * 512],
                        ones64[:], rowsb[:, ic * 512 : (ic + 1) * 512],
                        start=True, stop=True,
                        tile_position=(0, 64),
                        skip_group_check=True,
                    )

                aw_prev[0] = aw
            emit_app(3)

            for it in range(8):
                itsl = slice(it * 128, (it + 1) * 128)
                po = sps.tile([128, O], FP32, tag="s")
                for kt in range(4):
                    nc.tensor.matmul(
                        po[:], APP[kt][:, itsl], WOT[kt][:],
                        start=(kt == 0), stop=(kt == 3),
                    )
                ob = obpool.tile([128, O], FP32, tag="ob")
                nc.scalar.copy(ob[:], po[:])
                nc.scalar.dma_start(out[itsl, :], ob[:])


def build_nc(loop=0, use_bacc=False, unroll=1):
    cls = bacc.Bacc if use_bacc else bass.Bass
    nc = cls("TRN2", target_bir_lowering=False, debug=False, num_devices=N_CORES)
    xt = nc.declare_dram_parameter("xt", [C, N], F16, isOutput=False)
    wqkv = nc.declare_dram_parameter("wqkv", [C, 3 * HD], F16, isOutput=False)
    wot = nc.declare_dram_parameter("wot", [HD, O], F16, isOutput=False)
    out = nc.declare_dram_parameter("out", [N, O], FP32, isOutput=True)
    with _TC(nc, num_cores=N_CORES) as tc:
        if loop:
            with tc.For_i(0, loop, 1):
                _emit_body(tc, xt.ap(), wqkv.ap(), wot.ap(), out.ap())
        else:
            for _ in range(unroll):
                _emit_body(tc, xt.ap(), wqkv.ap(), wot.ap(), out.ap())
    return nc


def make_in_maps(features, weight_q, weight_k, weight_v, weight_out):
    wqkv = np.ascontiguousarray(
        np.concatenate(
            [
                weight_q.reshape(C, HD),
                weight_k.reshape(C, HD),
                weight_v.reshape(C, HD),
            ],
            axis=1,
        ),
        dtype=np.float16,
    )
    # attnw arrives scaled by 64 (iz = 64/Z); fold the 1/64 into Wout.
    wot = np.ascontiguousarray(
        weight_out.reshape(O, HD).T * (1.0 / 64.0), dtype=np.float16
    )
    in_maps = []
    for b in range(B):
        xt = np.ascontiguousarray(features[b].T, dtype=np.float16)
        in_maps.append({"xt": xt, "wqkv": wqkv, "wot": wot})
    return in_maps


_CACHED_NC = None


def kernel(features, weight_q, weight_k, weight_v, weight_out):
    global _CACHED_NC
    if _CACHED_NC is None:
        _CACHED_NC = build_nc(loop=0)
    in_maps = make_in_maps(
        np.asarray(features, np.float32),
        np.asarray(weight_q, np.float32),
        np.asarray(weight_k, np.float32),
        np.asarray(weight_v, np.float32),
        np.asarray(weight_out, np.float32),
    )
    res = run_bass_kernel_spmd(_CACHED_NC, in_maps, list(range(N_CORES)))
    return np.stack([res.results[b]["out"] for b in range(B)], axis=0)


if __name__ == "__main__":
    rng = np.random.default_rng(0)
    feats = rng.standard_normal((B, N, C)).astype(np.float32)
    wq = rng.standard_normal((C, H, D)).astype(np.float32) * 0.05
    wk = rng.standard_normal((C, H, D)).astype(np.float32) * 0.05
    wv = rng.standard_normal((C, H, D)).astype(np.float32) * 0.05
    wo = rng.standard_normal((O, H, D)).astype(np.float32) * 0.05
    o = kernel(feats, wq, wk, wv, wo)
    print("kernel ran, out shape", o.shape, "finite:", np.isfinite(o).all())
